# revision 9
# baseline (speedup 1.0000x reference)
"""Bahdanau attention TRN2 kernel.

Reference math (per batch b):
    qe = query @ W1 + b1                       # [Tq, U]
    ve = values @ W2 + b2                      # [Tv, U]
    score[q, v] = sum_u V[u] * tanh(qe[q, u] + ve[v, u])   (+ bV, dropped:
                  softmax over v is shift-invariant and score is not an
                  output, so bV cancels exactly)
    attn = softmax(score, axis=v)
    context = attn @ values

Sharding: 8 cores = 4 batches x 2 halves of Tq; softmax over Tv is local.

Per-core dataflow (U=256 on partitions as 2 chunks of 128):
  - Constants (identity, ones, b1/b2, V columns, W1, W2) are packed
    host-side into one [128, x] array loaded with two DMAs.
  - PE transposes values/query (identity matmul), projects ve_T[u, v] then
    qe_T[u, q]; b1+b2 folded into qe_T via a K=2 ones-matmul (fp32).
  - Broadcast-add + tanh, engine-balanced per measured rates (DVE
    tensor_scalar 263ns/op, ACT grouped tanh ~230ns/q-chunk, ACT direct
    fused bias+tanh 491ns/op): most q's are staged by DVE into S and
    tanh'd in one large-free-dim ACT op; DIRECT_Q q's per group go through
    ACT's fused path.  tanh output H is MM_DTYPE for the PE reduction.
  - V-reduction on PE: [K=128, M=1, N=512] matmuls (2 queries each),
    accumulated over the 2 U-chunks into PSUM (all chunk-0 passes emitted
    before chunk-1 so independent matmuls pipeline).  tile_position
    col-tiling spreads rows over partitions {0,32,64,96} x 8 free slots of
    a 4-bank PSUM tile (32 q's); a full-tile copy (DVE/ACT) moves it to
    SBUF and an SBUF->SBUF DMA regathers score[q, v] rows.
  - Softmax + context are pipelined per 32-query slice: ACT Exp with
    fused accum_out row-sum, DVE reciprocal + tensor_scalar mul, PE
    transpose of the attn slice (row/col tile_position r0), two
    accumulating matmuls against values into ctx PSUM rows [r0, r0+32).
"""

from contextlib import ExitStack

import numpy as np

import concourse.tile as tile
from concourse import bacc, mybir
from concourse.bass_utils import run_bass_kernel_spmd

F32 = mybir.dt.float32
BF16 = mybir.dt.bfloat16

N_CORES = 8
B, TQ, TV = 4, 256, 256
H, D, U = 512, 512, 256
TQS = TQ // 2          # 128 queries per core
P = 128
UC = U // P            # 2 partition chunks of U
G = 8                  # queries per tanh group
NG = TQS // G
QPT = 16               # queries per PSUM score tile (2 banks)
GPT = QPT // G         # groups per score tile

# engine-balance tunables
DIRECT_Q = 1           # q's per group routed via ACT fused bias+tanh
STAGE_ON_ACT = 2       # of the 4 psum->sbuf score copies, how many on ACT
MM_DTYPE = "bf16"      # V-reduction dtype: bf16 | f32 | f32r

# wpack column layout (host-side packing of constants)
_IDENT_O = 0
_ONES_O = _IDENT_O + P        # rows 0-1 used
_B12_O = _ONES_O + P          # rows 0-1 used: b1 | b2
_VCOL_O = _B12_O + U
_W1_O = _VCOL_O + UC
_W2_O = _W1_O + 4 * U
_WPACK_COLS = _W2_O + 4 * U

_PROGRAM = None


def _build_program():
    nc = bacc.Bacc("TRN2", target_bir_lowering=False, debug=False,
                   num_devices=N_CORES)

    q_in = nc.declare_dram_parameter("q", [TQS, H], F32, isOutput=False)
    v_in = nc.declare_dram_parameter("v", [TV, D], F32, isOutput=False)
    wp_in = nc.declare_dram_parameter("wpack", [P, _WPACK_COLS], F32,
                                      isOutput=False)
    ctx_out = nc.declare_dram_parameter("ctx", [TQS, D], F32, isOutput=True)
    attn_out = nc.declare_dram_parameter("attn", [TQS, TV], F32, isOutput=True)

    h_dt = {"bf16": BF16, "f32": F32, "f32r": F32}[MM_DTYPE]

    def mm_ap(ap):
        return ap.bitcast(mybir.dt.float32r) if MM_DTYPE == "f32r" else ap

    with tile.TileContext(nc) as tc, ExitStack() as octx:
        consts = octx.enter_context(tc.tile_pool(name="consts", bufs=1))
        work = octx.enter_context(tc.tile_pool(name="work", bufs=1))
        s_pool = octx.enter_context(tc.tile_pool(name="s", bufs=3))
        stage_pool = octx.enter_context(tc.tile_pool(name="stage", bufs=2))

        # ---- inputs ----
        wp = consts.tile([P, _WPACK_COLS], F32, name="wp", tag="wp")
        half = _WPACK_COLS // 2
        nc.sync.dma_start(wp[:, 0:half], wp_in[:, 0:half])
        nc.sync.dma_start(wp[:, half:], wp_in[:, half:])
        ident = wp[:, _IDENT_O:_IDENT_O + P]
        ones2 = wp[0:2, _ONES_O:_ONES_O + P]
        b12 = wp[0:2, _B12_O:_B12_O + U]
        v_col = wp[:, _VCOL_O:_VCOL_O + UC]
        w1_sb = wp[:, _W1_O:_W1_O + 4 * U]
        w2_sb = wp[:, _W2_O:_W2_O + 4 * U]

        values_big = consts.tile([P, 2 * D], F32, name="values", tag="values")
        nc.sync.dma_start(values_big[:].rearrange("p (a d) -> p a d", a=2),
                          v_in[:].rearrange("(a p) d -> p a d", p=P))
        values_sb = [values_big[:, a * D:(a + 1) * D] for a in range(2)]
        query_sb = consts.tile([P, H], F32, name="query", tag="query")
        nc.sync.dma_start(query_sb[:], q_in[:])

        if MM_DTYPE == "bf16":
            v_col_mm = consts.tile([P, UC], BF16, name="vcol_mm", tag="vcol_mm")
            nc.vector.tensor_copy(v_col_mm[:], v_col)
        else:
            v_col_mm = v_col

        qe = [consts.tile([P, TQS], F32, name=f"qe{c}", tag=f"qe{c}")
              for c in range(UC)]
        ve = [consts.tile([P, TV], F32, name=f"ve{c}", tag=f"ve{c}")
              for c in range(UC)]

        with tc.tile_pool(name="ph1_ps", bufs=2, space="PSUM") as ph1_ps:
            # values chain first: it gates the first tanh group
            vT = consts.tile([P, 4 * TV], F32, name="vT", tag="vT")
            for j in range(4):
                for a in range(2):
                    ps = ph1_ps.tile([P, P], F32, name="tp", tag="tp")
                    nc.tensor.transpose(ps[:], values_sb[a][:, j * P:(j + 1) * P],
                                        ident)
                    nc.vector.tensor_copy(
                        vT[:, j * TV + a * P:j * TV + (a + 1) * P], ps[:])
            for c in range(UC):
                ps2 = ph1_ps.tile([P, TV], F32, name="ve_ps", tag="ve_ps")
                for k in range(4):
                    nc.tensor.matmul(ps2[:],
                                     w2_sb[:, k * U + c * P:k * U + (c + 1) * P],
                                     vT[:, k * TV:(k + 1) * TV],
                                     start=(k == 0), stop=(k == 3))
                nc.vector.tensor_copy(ve[c][:], ps2[:])

            qT = consts.tile([P, H], F32, name="qT", tag="qT")
            for j in range(4):
                ps = ph1_ps.tile([P, P], F32, name="tp", tag="tp")
                nc.tensor.transpose(ps[:], query_sb[:, j * P:(j + 1) * P], ident)
                nc.vector.tensor_copy(qT[:, j * P:(j + 1) * P], ps[:])
            for c in range(UC):
                ps = ph1_ps.tile([P, TQS], F32, name="qe_ps", tag="qe_ps")
                for k in range(4):
                    nc.tensor.matmul(ps[:],
                                     w1_sb[:, k * U + c * P:k * U + (c + 1) * P],
                                     qT[:, k * P:(k + 1) * P],
                                     start=(k == 0), stop=False)
                nc.tensor.matmul(ps[:], b12[:, c * P:(c + 1) * P], ones2,
                                 start=False, stop=True)
                nc.vector.tensor_copy(qe[c][:], ps[:])

        # ---- main loop ----
        n_grouped = G - DIRECT_Q
        score_sb = work.tile([P, TV], F32, name="score", tag="score")
        escore = work.tile([P, TV], F32, name="escore", tag="escore")
        rowsum = work.tile([P, 1], F32, name="rowsum", tag="rowsum")
        rinv = work.tile([P, 1], F32, name="rinv", tag="rinv")
        attn_sb = work.tile([P, TV], F32, name="attn", tag="attn")
        ctx_sb = work.tile([P, D], F32, name="ctx_sb", tag="ctx_sb")
        stage_k = 0
        with tc.tile_pool(name="score_ps", bufs=2, space="PSUM") as score_ps_pool, \
             tc.tile_pool(name="tail_ps", bufs=2, space="PSUM") as tail_ps, \
             tc.tile_pool(name="ctx_ps_pool", bufs=1, space="PSUM") as ctx_ps_pool:
            ctx_ps = ctx_ps_pool.tile([P, D], F32, name="ctx_ps", tag="ctx_ps")
            sps = None
            for g in range(NG):
                s_t = [s_pool.tile([P, n_grouped * TV], F32, name=f"s{c}",
                                   tag=f"s{c}") for c in range(UC)]
                h_t = [s_pool.tile([P, G * TV], h_dt, name=f"h{c}",
                                   tag=f"h{c}") for c in range(UC)]
                for i in range(n_grouped):
                    q = g * G + i
                    for c in range(UC):
                        nc.vector.tensor_scalar_add(
                            s_t[c][:, i * TV:(i + 1) * TV], ve[c][:],
                            qe[c][:, q:q + 1])
                for c in range(UC):
                    nc.scalar.activation(h_t[c][:, 0:n_grouped * TV], s_t[c][:],
                                         mybir.ActivationFunctionType.Tanh)
                for i in range(n_grouped, G):
                    q = g * G + i
                    for c in range(UC):
                        nc.scalar.activation(h_t[c][:, i * TV:(i + 1) * TV],
                                             ve[c][:],
                                             mybir.ActivationFunctionType.Tanh,
                                             bias=qe[c][:, q:q + 1])
                if g % GPT == 0:
                    sps = score_ps_pool.tile([P, QPT * 64], F32, name="sps", tag="sps")
                for c in range(UC):          # all chunk-0 passes, then chunk-1
                    for i in range(0, G, 2):
                        w = (g % GPT) * G + i
                        pos, slot = 32 * (w // (QPT // 4)), w % (QPT // 4)
                        nc.tensor.matmul(
                            sps[pos:pos + 1, slot * TV:(slot + 2) * TV],
                            mm_ap(v_col_mm[:, c:c + 1]),
                            mm_ap(h_t[c][:, i * TV:(i + 2) * TV]),
                            start=(c == 0), stop=(c == UC - 1),
                            tile_position=(0, pos))
                if g % GPT == GPT - 1:
                    # PSUM -> SBUF (full-tile copy), then SBUF -> SBUF DMA
                    # regathers rows {0,32,64,96} x slots into QPT score rows.
                    stg = stage_pool.tile([P, QPT * 64], F32, name="stg", tag="stg")
                    if stage_k % 2 == (0 if stage_k // 2 < STAGE_ON_ACT else 2):
                        nc.scalar.copy(stg[:], sps[:])
                    else:
                        nc.vector.tensor_copy(stg[:], sps[:])
                    stage_k += 1
                    t0 = (g - GPT + 1) * G
                    src = stg[:].rearrange("(a b) f -> a b f", b=32)[:, 0, :]
                    nc.sync.dma_start(score_sb[t0:t0 + QPT, :], src)
                    if t0 % 32 != 16:
                        continue
                    r0 = t0 - 16

                    # ---- pipelined softmax + context for rows [r0, r0+32) ----
                    sl = slice(r0, r0 + 32)
                    nc.scalar.activation(escore[sl, :], score_sb[sl, :],
                                         mybir.ActivationFunctionType.Exp,
                                         accum_out=rowsum[sl, :])
                    nc.vector.reciprocal(rinv[sl, :], rowsum[sl, :])
                    nc.vector.tensor_scalar_mul(attn_sb[sl, :], escore[sl, :],
                                                rinv[sl, :])
                    nc.sync.dma_start(attn_out[sl, :], attn_sb[sl, :])
                    for a in range(2):
                        ps = tail_ps.tile([P, 32], F32, name="tp2", tag="tp2")
                        nc.tensor.transpose(ps[:], attn_sb[sl, a * P:(a + 1) * P],
                                            ident[r0:r0 + 32, r0:r0 + 32],
                                            tile_position=(r0, 0))
                        attnT = work.tile([P, 32], F32, name="attnT",
                                          tag=f"attnT{(r0 // 32) % 2}")
                        nc.vector.tensor_copy(attnT[:], ps[:])
                        nc.tensor.matmul(ctx_ps[sl, :], attnT[:], values_sb[a],
                                         start=(a == 0), stop=(a == 1),
                                         tile_position=(0, r0))
                    nc.vector.tensor_copy(ctx_sb[sl, :], ctx_ps[sl, :])
                    nc.sync.dma_start(ctx_out[sl, :], ctx_sb[sl, :])

    nc.finalize()
    return nc


def _get_program():
    global _PROGRAM
    if _PROGRAM is None:
        _PROGRAM = _build_program()
    return _PROGRAM


TRACE = False
RUN_KWARGS = {}
LAST_RESULT = None


def _make_wpack(W1, W2, b1, b2, vv):
    wp = np.zeros((P, _WPACK_COLS), dtype=np.float32)
    wp[:, _IDENT_O:_IDENT_O + P] = np.eye(P, dtype=np.float32)
    wp[0:2, _ONES_O:_ONES_O + P] = 1.0
    wp[0, _B12_O:_B12_O + U] = b1
    wp[1, _B12_O:_B12_O + U] = b2
    for c in range(UC):
        wp[:, _VCOL_O + c] = vv[c * P:(c + 1) * P]
    for k in range(4):
        wp[:, _W1_O + k * U:_W1_O + (k + 1) * U] = W1[k * P:(k + 1) * P, :]
        wp[:, _W2_O + k * U:_W2_O + (k + 1) * U] = W2[k * P:(k + 1) * P, :]
    return wp


def kernel(query, values, W1, b1, W2, b2, V, bV):
    global LAST_RESULT
    query = np.ascontiguousarray(np.asarray(query, dtype=np.float32))
    values = np.ascontiguousarray(np.asarray(values, dtype=np.float32))
    vv = np.asarray(V, dtype=np.float32).reshape(U)
    wpack = _make_wpack(np.asarray(W1, dtype=np.float32),
                        np.asarray(W2, dtype=np.float32),
                        np.asarray(b1, dtype=np.float32),
                        np.asarray(b2, dtype=np.float32), vv)
    # bV shifts every score equally; softmax is shift-invariant and score is
    # not returned, so it has no effect on either output.

    nc = _get_program()
    in_maps = []
    for core in range(N_CORES):
        b, half = divmod(core, 2)
        in_maps.append({
            "q": np.ascontiguousarray(query[b, half * TQS:(half + 1) * TQS, :]),
            "v": values[b],
            "wpack": wpack,
        })

    res = run_bass_kernel_spmd(nc, in_maps, list(range(N_CORES)), trace=TRACE,
                               **RUN_KWARGS)
    LAST_RESULT = res

    context = np.empty((B, TQ, D), dtype=np.float32)
    attn = np.empty((B, TQ, TV, 1), dtype=np.float32)
    for core in range(N_CORES):
        b, half = divmod(core, 2)
        sl = slice(half * TQS, (half + 1) * TQS)
        context[b, sl, :] = res.results[core]["ctx"]
        attn[b, sl, :, 0] = res.results[core]["attn"]
    return context, attn


# revision 12
# speedup vs baseline: 1.0294x; 1.0294x over previous
"""Bahdanau attention TRN2 kernel.

Reference math (per batch b):
    qe = query @ W1 + b1                       # [Tq, U]
    ve = values @ W2 + b2                      # [Tv, U]
    score[q, v] = sum_u V[u] * tanh(qe[q, u] + ve[v, u])   (+ bV, dropped:
                  softmax over v is shift-invariant and score is not an
                  output, so bV cancels exactly)
    attn = softmax(score, axis=v)
    context = attn @ values

Sharding: 8 cores = 4 batches x 2 halves of Tq; softmax over Tv is local.

Per-core dataflow (U=256 on partitions as 2 chunks of 128):
  - Constants (identity, ones, b1/b2, V columns, W1, W2) are packed
    host-side into one [128, x] array loaded with two DMAs.
  - PE transposes values/query (identity matmul), projects ve_T[u, v] then
    qe_T[u, q]; b1+b2 folded into qe_T via a K=2 ones-matmul (fp32).
  - Broadcast-add + tanh, engine-balanced per measured rates (DVE
    tensor_scalar 263ns/op, ACT grouped tanh ~230ns/q-chunk, ACT direct
    fused bias+tanh 491ns/op): most q's are staged by DVE into S and
    tanh'd in one large-free-dim ACT op; DIRECT_Q q's per group go through
    ACT's fused path.  tanh output H is MM_DTYPE for the PE reduction.
  - V-reduction on PE: [K=128, M=1, N=512] matmuls (2 queries each),
    accumulated over the 2 U-chunks into PSUM (all chunk-0 passes emitted
    before chunk-1 so independent matmuls pipeline).  tile_position
    col-tiling spreads rows over partitions {0,32,64,96} x 8 free slots of
    a 4-bank PSUM tile (32 q's); a full-tile copy (DVE/ACT) moves it to
    SBUF and an SBUF->SBUF DMA regathers score[q, v] rows.
  - Softmax + context are pipelined per 32-query slice: ACT Exp with
    fused accum_out row-sum, DVE reciprocal + tensor_scalar mul, PE
    transpose of the attn slice (row/col tile_position r0), two
    accumulating matmuls against values into ctx PSUM rows [r0, r0+32).
"""

from contextlib import ExitStack

import numpy as np

import concourse.tile as tile
from concourse import bacc, mybir
from concourse.bass_utils import run_bass_kernel_spmd

F32 = mybir.dt.float32
BF16 = mybir.dt.bfloat16

N_CORES = 8
B, TQ, TV = 4, 256, 256
H, D, U = 512, 512, 256
TQS = TQ // 2          # 128 queries per core
P = 128
UC = U // P            # 2 partition chunks of U
G = 16                 # queries per tanh group
NG = TQS // G
QPT = 16               # queries per PSUM score tile (2 banks)
GPT = QPT // G         # groups per score tile

# engine-balance tunables
DIRECT_Q = 1           # q's per group routed via ACT fused bias+tanh
STAGE_ON_ACT = 2       # of every 2 psum->sbuf score copies, how many on ACT
MM_DTYPE = "bf16"      # V-reduction dtype: bf16 | f32 | f32r

# wpack column layout (host-side packing of constants)
_IDENT_O = 0
_ONES_O = _IDENT_O + P        # rows 0-1 used
_B12_O = _ONES_O + P          # rows 0-1 used: b1 | b2
_VCOL_O = _B12_O + U
_W1_O = _VCOL_O + UC
_W2_O = _W1_O + 4 * U
_WPACK_COLS = _W2_O + 4 * U

_PROGRAM = None


def _build_program():
    nc = bacc.Bacc("TRN2", target_bir_lowering=False, debug=False,
                   num_devices=N_CORES)

    q_in = nc.declare_dram_parameter("q", [TQS, H], F32, isOutput=False)
    v_in = nc.declare_dram_parameter("v", [TV, D], F32, isOutput=False)
    wp_in = nc.declare_dram_parameter("wpack", [P, _WPACK_COLS], F32,
                                      isOutput=False)
    ctx_out = nc.declare_dram_parameter("ctx", [TQS, D], F32, isOutput=True)
    attn_out = nc.declare_dram_parameter("attn", [TQS, TV], F32, isOutput=True)

    h_dt = {"bf16": BF16, "f32": F32, "f32r": F32}[MM_DTYPE]

    def mm_ap(ap):
        return ap.bitcast(mybir.dt.float32r) if MM_DTYPE == "f32r" else ap

    with tile.TileContext(nc) as tc, ExitStack() as octx:
        consts = octx.enter_context(tc.tile_pool(name="consts", bufs=1))
        work = octx.enter_context(tc.tile_pool(name="work", bufs=1))
        s_pool = octx.enter_context(tc.tile_pool(name="s", bufs=3))
        stage_pool = octx.enter_context(tc.tile_pool(name="stage", bufs=2))

        score_sb = work.tile([P, TV], F32, name="score", tag="score")

        # ---- inputs ----
        wp = consts.tile([P, _WPACK_COLS], F32, name="wp", tag="wp")
        half = _WPACK_COLS // 2
        nc.sync.dma_start(wp[:, 0:half], wp_in[:, 0:half])
        nc.sync.dma_start(wp[:, half:], wp_in[:, half:])
        ident = wp[:, _IDENT_O:_IDENT_O + P]
        ones2 = wp[0:2, _ONES_O:_ONES_O + P]
        b12 = wp[0:2, _B12_O:_B12_O + U]
        v_col = wp[:, _VCOL_O:_VCOL_O + UC]
        w1_sb = wp[:, _W1_O:_W1_O + 4 * U]
        w2_sb = wp[:, _W2_O:_W2_O + 4 * U]

        # a few throwaway matmuls warm the PE clock (HAM) during DMA wait;
        # the result is parked in score_sb (fully overwritten later)
        with tc.tile_pool(name="warm_ps", bufs=1, space="PSUM") as warm_ps:
            wps = warm_ps.tile([P, P], F32, name="wps", tag="wps")
            for _ in range(10):
                nc.tensor.matmul(wps[:], ident, ident, start=True, stop=True)
            nc.vector.tensor_copy(score_sb[:, 0:P], wps[:])

        values_big = consts.tile([P, 2 * D], F32, name="values", tag="values")
        nc.sync.dma_start(values_big[:].rearrange("p (a d) -> p a d", a=2),
                          v_in[:].rearrange("(a p) d -> p a d", p=P))
        values_sb = [values_big[:, a * D:(a + 1) * D] for a in range(2)]
        query_sb = consts.tile([P, H], F32, name="query", tag="query")
        nc.sync.dma_start(query_sb[:], q_in[:])

        if MM_DTYPE == "bf16":
            v_col_mm = consts.tile([P, UC], BF16, name="vcol_mm", tag="vcol_mm")
            nc.vector.tensor_copy(v_col_mm[:], v_col)
        else:
            v_col_mm = v_col

        qe = [consts.tile([P, TQS], F32, name=f"qe{c}", tag=f"qe{c}")
              for c in range(UC)]
        ve = [consts.tile([P, TV], F32, name=f"ve{c}", tag=f"ve{c}")
              for c in range(UC)]

        with tc.tile_pool(name="ph1_ps", bufs=2, space="PSUM") as ph1_ps:
            # values chain first: it gates the first tanh group
            vT = consts.tile([P, 4 * TV], F32, name="vT", tag="vT")
            for j in range(4):
                for a in range(2):
                    ps = ph1_ps.tile([P, P], F32, name="tp", tag="tp")
                    nc.tensor.transpose(ps[:], values_sb[a][:, j * P:(j + 1) * P],
                                        ident)
                    nc.vector.tensor_copy(
                        vT[:, j * TV + a * P:j * TV + (a + 1) * P], ps[:])
            for c in range(UC):
                ps2 = ph1_ps.tile([P, TV], F32, name="ve_ps", tag="ve_ps")
                for k in range(4):
                    nc.tensor.matmul(ps2[:],
                                     w2_sb[:, k * U + c * P:k * U + (c + 1) * P],
                                     vT[:, k * TV:(k + 1) * TV],
                                     start=(k == 0), stop=(k == 3))
                nc.vector.tensor_copy(ve[c][:], ps2[:])

            qT = consts.tile([P, H], F32, name="qT", tag="qT")
            for j in range(4):
                ps = ph1_ps.tile([P, P], F32, name="tp", tag="tp")
                nc.tensor.transpose(ps[:], query_sb[:, j * P:(j + 1) * P], ident)
                nc.vector.tensor_copy(qT[:, j * P:(j + 1) * P], ps[:])
            for c in range(UC):
                ps = ph1_ps.tile([P, TQS], F32, name="qe_ps", tag="qe_ps")
                for k in range(4):
                    nc.tensor.matmul(ps[:],
                                     w1_sb[:, k * U + c * P:k * U + (c + 1) * P],
                                     qT[:, k * P:(k + 1) * P],
                                     start=(k == 0), stop=False)
                nc.tensor.matmul(ps[:], b12[:, c * P:(c + 1) * P], ones2,
                                 start=False, stop=True)
                nc.vector.tensor_copy(qe[c][:], ps[:])

        # ---- main loop ----
        n_grouped = G - DIRECT_Q
        escore = work.tile([P, TV], F32, name="escore", tag="escore")
        rowsum = work.tile([P, 1], F32, name="rowsum", tag="rowsum")
        rinv = work.tile([P, 1], F32, name="rinv", tag="rinv")
        attn_sb = work.tile([P, TV], F32, name="attn", tag="attn")
        ctx_sb = work.tile([P, D], F32, name="ctx_sb", tag="ctx_sb")
        stage_k = 0
        with tc.tile_pool(name="score_ps", bufs=2, space="PSUM") as score_ps_pool, \
             tc.tile_pool(name="tail_ps", bufs=2, space="PSUM") as tail_ps, \
             tc.tile_pool(name="ctx_ps_pool", bufs=1, space="PSUM") as ctx_ps_pool:
            ctx_ps = ctx_ps_pool.tile([P, D], F32, name="ctx_ps", tag="ctx_ps")
            sps = None
            for g in range(NG):
                s_t = [s_pool.tile([P, n_grouped * TV], F32, name=f"s{c}",
                                   tag=f"s{c}") for c in range(UC)]
                h_t = [s_pool.tile([P, G * TV], h_dt, name=f"h{c}",
                                   tag=f"h{c}") for c in range(UC)]
                for i in range(n_grouped):
                    q = g * G + i
                    for c in range(UC):
                        nc.vector.tensor_scalar_add(
                            s_t[c][:, i * TV:(i + 1) * TV], ve[c][:],
                            qe[c][:, q:q + 1])
                for c in range(UC):
                    nc.scalar.activation(h_t[c][:, 0:n_grouped * TV], s_t[c][:],
                                         mybir.ActivationFunctionType.Tanh)
                for i in range(n_grouped, G):
                    q = g * G + i
                    for c in range(UC):
                        nc.scalar.activation(h_t[c][:, i * TV:(i + 1) * TV],
                                             ve[c][:],
                                             mybir.ActivationFunctionType.Tanh,
                                             bias=qe[c][:, q:q + 1])
                if g % GPT == 0:
                    sps = score_ps_pool.tile([P, QPT * 64], F32, name="sps", tag="sps")
                for c in range(UC):          # all chunk-0 passes, then chunk-1
                    for i in range(0, G, 2):
                        w = (g % GPT) * G + i
                        pos, slot = 32 * (w // (QPT // 4)), w % (QPT // 4)
                        nc.tensor.matmul(
                            sps[pos:pos + 1, slot * TV:(slot + 2) * TV],
                            mm_ap(v_col_mm[:, c:c + 1]),
                            mm_ap(h_t[c][:, i * TV:(i + 2) * TV]),
                            start=(c == 0), stop=(c == UC - 1),
                            tile_position=(0, pos))
                if g % GPT == GPT - 1:
                    # PSUM -> SBUF (full-tile copy), then SBUF -> SBUF DMA
                    # regathers rows {0,32,64,96} x slots into QPT score rows.
                    stg = stage_pool.tile([P, QPT * 64], F32, name="stg", tag="stg")
                    if stage_k % 2 < STAGE_ON_ACT:
                        nc.scalar.copy(stg[:], sps[:])
                    else:
                        nc.vector.tensor_copy(stg[:], sps[:])
                    stage_k += 1
                    t0 = (g - GPT + 1) * G
                    src = stg[:].rearrange("(a b) f -> a b f", b=32)[:, 0, :]
                    nc.sync.dma_start(score_sb[t0:t0 + QPT, :], src)

                    if (t0 + QPT) % 32 != 0:
                        continue
                    # ---- pipelined softmax for rows [r0, r0+32) ----
                    r0 = t0 + QPT - 32
                    sl = slice(r0, r0 + 32)
                    nc.scalar.activation(escore[sl, :], score_sb[sl, :],
                                         mybir.ActivationFunctionType.Exp,
                                         accum_out=rowsum[sl, :])
                    nc.vector.reciprocal(rinv[sl, :], rowsum[sl, :])
                    nc.vector.tensor_scalar_mul(attn_sb[sl, :], escore[sl, :],
                                                rinv[sl, :])
                    nc.sync.dma_start(attn_out[sl, :], attn_sb[sl, :])

            # ---- context = attn @ values ----
            for a in range(2):
                ps = tail_ps.tile([P, P], F32, name="tp2", tag="tp2")
                nc.tensor.transpose(ps[:], attn_sb[:, a * P:(a + 1) * P], ident)
                attnT = work.tile([P, P], F32, name="attnT", tag=f"attnT{a}")
                nc.vector.tensor_copy(attnT[:], ps[:])
                nc.tensor.matmul(ctx_ps[:], attnT[:], values_sb[a],
                                 start=(a == 0), stop=(a == 1))
            nc.vector.tensor_copy(ctx_sb[:], ctx_ps[:])
            nc.sync.dma_start(ctx_out[:], ctx_sb[:])

    nc.finalize()
    return nc


def _get_program():
    global _PROGRAM
    if _PROGRAM is None:
        _PROGRAM = _build_program()
    return _PROGRAM


TRACE = False
RUN_KWARGS = {}
LAST_RESULT = None


def _make_wpack(W1, W2, b1, b2, vv):
    wp = np.zeros((P, _WPACK_COLS), dtype=np.float32)
    wp[:, _IDENT_O:_IDENT_O + P] = np.eye(P, dtype=np.float32)
    wp[0:2, _ONES_O:_ONES_O + P] = 1.0
    wp[0, _B12_O:_B12_O + U] = b1
    wp[1, _B12_O:_B12_O + U] = b2
    for c in range(UC):
        wp[:, _VCOL_O + c] = vv[c * P:(c + 1) * P]
    for k in range(4):
        wp[:, _W1_O + k * U:_W1_O + (k + 1) * U] = W1[k * P:(k + 1) * P, :]
        wp[:, _W2_O + k * U:_W2_O + (k + 1) * U] = W2[k * P:(k + 1) * P, :]
    return wp


def kernel(query, values, W1, b1, W2, b2, V, bV):
    global LAST_RESULT
    query = np.ascontiguousarray(np.asarray(query, dtype=np.float32))
    values = np.ascontiguousarray(np.asarray(values, dtype=np.float32))
    vv = np.asarray(V, dtype=np.float32).reshape(U)
    wpack = _make_wpack(np.asarray(W1, dtype=np.float32),
                        np.asarray(W2, dtype=np.float32),
                        np.asarray(b1, dtype=np.float32),
                        np.asarray(b2, dtype=np.float32), vv)
    # bV shifts every score equally; softmax is shift-invariant and score is
    # not returned, so it has no effect on either output.

    nc = _get_program()
    in_maps = []
    for core in range(N_CORES):
        b, half = divmod(core, 2)
        in_maps.append({
            "q": np.ascontiguousarray(query[b, half * TQS:(half + 1) * TQS, :]),
            "v": values[b],
            "wpack": wpack,
        })

    res = run_bass_kernel_spmd(nc, in_maps, list(range(N_CORES)), trace=TRACE,
                               **RUN_KWARGS)
    LAST_RESULT = res

    context = np.empty((B, TQ, D), dtype=np.float32)
    attn = np.empty((B, TQ, TV, 1), dtype=np.float32)
    for core in range(N_CORES):
        b, half = divmod(core, 2)
        sl = slice(half * TQS, (half + 1) * TQS)
        context[b, sl, :] = res.results[core]["ctx"]
        attn[b, sl, :, 0] = res.results[core]["attn"]
    return context, attn
